# revision 21
# baseline (speedup 1.0000x reference)
"""nn_ARPrior kernel for 8 TRN2 NeuronCores (data-parallel over batch).

Reference computation (per batch row b, latent index l):
    inp[b,l] = 0 if l==0 else mean(z[b,:l])
    h1 = relu(inp * W1[l,0,:] + b1[l])          # (128,)
    h2 = relu(h1 @ W2[l] + b2[l])               # (64,)
    out = h2 @ W3[l] + b3[l]                    # (2,) -> (mu, logvar)
Returns (mus, lvs), each (B, L) float32.

Mapping (per core, B_LOC = 4096 batch rows):
  - inp is linear in z: inp = z @ M with M[i,l] = 1/l for i < l else 0.
    Layer 1 for latent l is a single K=32 matmul with
    lhsT[k,h] = M[k,l]*W1[l,0,h] (k<=30) and lhsT[31,h] = b1[l,h], against a
    shared moving operand [z^T rows 0..30 ; ones] (z row 31 is never needed,
    M is strictly upper triangular).
  - The moving operand is stacked 4x across SBUF partitions so 4 latents run
    concurrently on the PE array via row-tiling (tile_position=(32s,0)).
  - Layer 2: per-latent [128,64] matmul; two latents run concurrently via
    column-tiling (tile_position=(0,0)/(0,64)) into one PSUM bank.
  - Layer 3: latent pairs are packed block-diagonally into [128,4] weights;
    4 pairs run concurrently via column-tiling (tile_position=(0,32p)).
  - All matmul inputs are bf16 (fp32 accumulate in PSUM). All PE matmuls are
    chained with ordering-only deps so rotation groups stay adjacent in the
    PE stream (adjacency is what makes tile_position concurrency engage).
  - The kernel is lane-bound: ScalarE+VectorE evacuating PSUM (h1/h2/out) at
    1 elem/lane/cycle is the floor; relu/bias stages are split across the
    two engines by a cost-balancing scheduler.
  - Outputs accumulate into four persistent [128, B_LOC] wall tiles whose
    column chunks DMA out as soon as each chunk's layer-3 completes.
"""

import numpy as np
import ml_dtypes

import concourse.bass as bass
import concourse.tile as tile
from concourse import bacc, mybir
from concourse.bass_utils import run_bass_kernel_spmd

B = 32768
L = 32
H1 = 128
H2 = 64
N_CORES = 8
B_LOC = B // N_CORES          # 4096 batch rows per core
NT = 512                      # columns per matmul (fp32 PSUM bank)
N_BT = B_LOC // NT            # 8 batch tiles
N_QUAD = L // 4               # 8 quads of 4 latents
N_PAIR = L // 2               # 16 latent pairs
N_WALL = N_PAIR // 4          # 4 walls of 4 pairs

BF16 = mybir.dt.bfloat16
F32 = mybir.dt.float32
NP_BF16 = ml_dtypes.bfloat16


def build_program():
    """Build the per-core bass program (identical on all 8 cores)."""
    nc = bacc.Bacc("TRN2", target_bir_lowering=False, debug=False,
                   num_devices=N_CORES)

    d_zt4 = nc.dram_tensor("zt4", [128, B_LOC], BF16, kind="ExternalInput")
    d_w1e = nc.dram_tensor("w1e", [128, N_QUAD * H1], BF16, kind="ExternalInput")
    d_w2 = nc.dram_tensor("w2", [128, L * H2], BF16, kind="ExternalInput")
    d_w3 = nc.dram_tensor("w3", [128, N_PAIR * 4], BF16, kind="ExternalInput")
    d_b2 = nc.dram_tensor("b2", [128, N_PAIR], F32, kind="ExternalInput")
    d_b3 = nc.dram_tensor("b3", [128, N_WALL], F32, kind="ExternalInput")
    d_out = nc.dram_tensor("out", [N_WALL, 128, B_LOC], F32,
                           kind="ExternalOutput")

    # Lane-engine load balancer: assign each relu/bias op to the engine with
    # less accumulated estimated time.  Costs in ns per op (HW-measured).
    lane_time = {"dve": 0.0, "act": 0.0}

    def pick_engine(dve_cost, act_cost):
        if lane_time["dve"] + dve_cost <= lane_time["act"] + act_cost:
            lane_time["dve"] += dve_cost
            return "dve"
        lane_time["act"] += act_cost
        return "act"

    with tile.TileContext(nc) as tc:
        with (
            tc.tile_pool(name="consts", bufs=1) as consts,
            tc.tile_pool(name="h1p", bufs=10) as h1p,
            tc.tile_pool(name="h2p", bufs=16) as h2p,
            tc.tile_pool(name="outp", bufs=1) as outp,
            tc.tile_pool(name="p1", bufs=2, space="PSUM") as p1,
            tc.tile_pool(name="p2", bufs=3, space="PSUM") as p2,
            tc.tile_pool(name="p3", bufs=1, space="PSUM") as p3,
        ):
            zt4 = consts.tile([128, B_LOC], BF16)
            w1e = consts.tile([128, N_QUAD * H1], BF16)
            w2 = consts.tile([128, L * H2], BF16)
            w3 = consts.tile([128, N_PAIR * 4], BF16)
            b2 = consts.tile([128, N_PAIR], F32)
            b3 = consts.tile([128, N_WALL], F32)
            nc.gpsimd.dma_start(out=w1e[:], in_=d_w1e[:])
            for _c in range(8):
                _sl = slice(_c * (B_LOC // 8), (_c + 1) * (B_LOC // 8))
                nc.gpsimd.dma_start(out=zt4[:, _sl], in_=d_zt4[:, _sl])
            nc.gpsimd.dma_start(out=w2[:], in_=d_w2[:])
            nc.gpsimd.dma_start(out=w3[:], in_=d_w3[:])
            nc.gpsimd.dma_start(out=b2[:], in_=d_b2[:])
            nc.gpsimd.dma_start(out=b3[:], in_=d_b3[:])

            # Pre-warm the ACT relu table set so its ~2.7us load overlaps
            # the input DMAs instead of delaying the first real relu.
            warm = consts.tile([1, 8], F32)
            nc.vector.memset(warm[:], 0.0)
            nc.scalar.activation(out=warm[:], in_=warm[:],
                                 func=mybir.ActivationFunctionType.Relu)

            def relu_from_psum(dst, src, force=None):
                # plain relu, PSUM(f32) -> SBUF(bf16), FD = 1024
                if force is None:
                    eng = pick_engine(dve_cost=1180.0, act_cost=1000.0)
                else:
                    eng = force
                    lane_time[eng] += 1180.0 if eng == "dve" else 1000.0
                if eng == "dve":
                    nc.vector.tensor_scalar(
                        out=dst, in0=src, scalar1=0.0, scalar2=None,
                        op0=mybir.AluOpType.max)
                else:
                    nc.scalar.activation(
                        out=dst, in_=src,
                        func=mybir.ActivationFunctionType.Relu)
                return eng

            def relu_bias_from_psum(dst, src, bias_ap):
                # relu(x + bias), PSUM(f32) -> SBUF(bf16), FD = 512
                eng = pick_engine(dve_cost=658.0, act_cost=570.0)
                if eng == "dve":
                    nc.vector.tensor_scalar(
                        out=dst, in0=src, scalar1=bias_ap, scalar2=0.0,
                        op0=mybir.AluOpType.add, op1=mybir.AluOpType.max)
                else:
                    nc.scalar.activation(
                        out=dst, in_=src,
                        func=mybir.ActivationFunctionType.Relu,
                        bias=bias_ap, scale=1.0)

            def bias_from_psum(dst, src, bias_ap):
                # x + bias, PSUM(f32) -> SBUF(f32), FD = 512
                # ACT's biased-Identity measured erratic; keep on DVE.
                eng = pick_engine(dve_cost=671.0, act_cost=100000.0)
                if eng == "dve":
                    nc.vector.tensor_scalar(
                        out=dst, in0=src, scalar1=bias_ap, scalar2=None,
                        op0=mybir.AluOpType.add)
                else:
                    nc.scalar.activation(
                        out=dst, in_=src,
                        func=mybir.ActivationFunctionType.Identity,
                        bias=bias_ap, scale=1.0)

            # Chain all PE matmuls with ordering-only deps so the scheduler
            # keeps rotation groups adjacent in the PE stream.
            pe_state = {"last": None}

            def mm(out, lhsT, rhs, tp):
                inst = nc.tensor.matmul(
                    out=out, lhsT=lhsT, rhs=rhs, start=True, stop=True,
                    tile_position=tp)
                if pe_state["last"] is not None:
                    bass._add_dep_helper(
                        inst.ins, pe_state["last"].ins, sync=False,
                        reason="pe-order")
                pe_state["last"] = inst

            def emit_l1(q, t, h1_tiles):
                col = slice(t * NT, (t + 1) * NT)
                ps_a = p1.tile([128, 2 * NT], F32, tag="p1", name=f"p1a_{q}_{t}")
                ps_b = p1.tile([128, 2 * NT], F32, tag="p1", name=f"p1b_{q}_{t}")
                for s in range(4):
                    ps = ps_a if s < 2 else ps_b
                    half = slice((s % 2) * NT, (s % 2) * NT + NT)
                    mm(ps[:, half],
                       w1e[32 * s:32 * s + 32, q * H1:(q + 1) * H1],
                       zt4[32 * s:32 * s + 32, col],
                       (32 * s, 0))
                h1_a = h1p.tile([128, 2 * NT], BF16, tag="h1", name=f"h1a_{q}_{t}")
                h1_b = h1p.tile([128, 2 * NT], BF16, tag="h1", name=f"h1b_{q}_{t}")
                # drain the quad's two PSUM tiles on DIFFERENT engines so
                # they evacuate concurrently and L2 unblocks sooner
                eng_a = relu_from_psum(h1_a[:], ps_a[:])
                relu_from_psum(h1_b[:], ps_b[:],
                               force=("act" if eng_a == "dve" else "dve"))
                h1_tiles[(q, t)] = (h1_a, h1_b)

            def emit_l2(q, t, h1_tiles, h2_tiles):
                h1_a, h1_b = h1_tiles.pop((q, t))
                for jj in range(2):
                    j = 2 * q + jj            # global pair index
                    h1t = h1_a if jj == 0 else h1_b
                    ps2 = p2.tile([128, NT], F32, tag="p2", name=f"p2_{j}_{t}")
                    for u in range(2):        # latent l = 2j + u
                        lat = 2 * j + u
                        mm(ps2[64 * u:64 * u + 64, :],
                           w2[:, H2 * lat:H2 * (lat + 1)],
                           h1t[:, u * NT:(u + 1) * NT],
                           (0, 64 * u))
                    h2t = h2p.tile([128, NT], BF16, tag="h2", name=f"h2_{j}_{t}")
                    relu_bias_from_psum(h2t[:], ps2[:], b2[:, j:j + 1])
                    h2_tiles[(j, t)] = h2t

            def emit_l3(w, t, h2_tiles, wall_tiles):
                col = slice(t * NT, (t + 1) * NT)
                ps3 = p3.tile([128, NT], F32, tag="p3", name=f"p3_{w}_{t}")
                for p in range(4):
                    j = 4 * w + p
                    mm(ps3[32 * p:32 * p + 4, :],
                       w3[:, 4 * j:4 * j + 4],
                       h2_tiles.pop((j, t))[:],
                       (0, 32 * p))
                bias_from_psum(wall_tiles[w][:, col], ps3[:], b3[:, w:w + 1])
                nc.gpsimd.dma_start(out=d_out[w, :, col],
                                    in_=wall_tiles[w][:, col])

            # Software-pipelined emission: L2 lags L1 by one step, L3 lags L2.
            wall_tiles = [
                outp.tile([128, B_LOC], F32, name=f"wall{w}")
                for w in range(N_WALL)
            ]
            steps = [(q, t) for t in range(N_BT) for q in range(N_QUAD)]
            h1_tiles, h2_tiles = {}, {}
            prev = None          # (q, t) whose L2 is pending
            prev_l3 = None       # (w, t) whose L3 is pending
            for (q, t) in steps:
                emit_l1(q, t, h1_tiles)
                if prev is not None:
                    pq, pt = prev
                    emit_l2(pq, pt, h1_tiles, h2_tiles)
                    if prev_l3 is not None:
                        emit_l3(*prev_l3, h2_tiles, wall_tiles)
                        prev_l3 = None
                    if pq % 2 == 1:
                        prev_l3 = (pq // 2, pt)
                prev = (q, t)
            pq, pt = prev
            emit_l2(pq, pt, h1_tiles, h2_tiles)
            if prev_l3 is not None:
                emit_l3(*prev_l3, h2_tiles, wall_tiles)
            emit_l3(pq // 2, pt, h2_tiles, wall_tiles)

    nc.compile()
    return nc


def marshal_inputs(z, W1, b1, W2, b2, W3, b3):
    """Build the 8 per-core input maps from full fp32 inputs."""
    z = np.asarray(z, dtype=np.float32)
    W1 = np.asarray(W1, dtype=np.float64)
    b1 = np.asarray(b1, dtype=np.float64)
    W2 = np.asarray(W2, dtype=np.float32)
    b2 = np.asarray(b2, dtype=np.float32)
    W3 = np.asarray(W3, dtype=np.float32)
    b3 = np.asarray(b3, dtype=np.float32)

    # Cumsum/mean fold: M[i,l] = 1/l for i < l else 0 (col 0 = zeros).
    M = np.zeros((L, L), dtype=np.float64)
    for l in range(1, L):
        M[:l, l] = 1.0 / l

    # W1eff[l, k, h]: k<=30 -> M[k,l] * W1[l,0,h]; k==31 -> b1[l,h].
    w1eff = np.einsum("kl,lh->lkh", M, W1[:, 0, :])   # (L, 32, 128)
    w1eff[:, 31, :] = b1                              # row 31 of M is all zero
    # pack: w1e[32s+k, 128q+h] = w1eff[4q+s, k, h]
    w1e = np.zeros((128, N_QUAD * H1), dtype=np.float64)
    for q in range(N_QUAD):
        for s in range(4):
            w1e[32 * s:32 * s + 32, q * H1:(q + 1) * H1] = w1eff[4 * q + s]
    w1e = w1e.astype(NP_BF16)

    # w2[h, 64l+o] = W2[l, h, o]
    w2sb = np.transpose(W2, (1, 0, 2)).reshape(H1, L * H2).astype(NP_BF16)

    # w3 block-diag pairs: [128, 4 per pair]
    w3sb = np.zeros((128, N_PAIR * 4), dtype=np.float32)
    for j in range(N_PAIR):
        w3sb[0:64, 4 * j + 0] = W3[2 * j, :, 0]
        w3sb[0:64, 4 * j + 1] = W3[2 * j, :, 1]
        w3sb[64:128, 4 * j + 2] = W3[2 * j + 1, :, 0]
        w3sb[64:128, 4 * j + 3] = W3[2 * j + 1, :, 1]
    w3sb = w3sb.astype(NP_BF16)

    # b2sb[o, j] = b2[2j, o]; b2sb[64+o, j] = b2[2j+1, o]
    b2sb = np.zeros((128, N_PAIR), dtype=np.float32)
    for j in range(N_PAIR):
        b2sb[0:64, j] = b2[2 * j]
        b2sb[64:128, j] = b2[2 * j + 1]

    # b3sb[32p + (2*wl + m), w] = b3[8w + 2p + wl, m]
    b3sb = np.zeros((128, N_WALL), dtype=np.float32)
    for w in range(N_WALL):
        for p in range(4):
            for wl in range(2):
                lat = 8 * w + 2 * p + wl
                b3sb[32 * p + 2 * wl + 0, w] = b3[lat, 0]
                b3sb[32 * p + 2 * wl + 1, w] = b3[lat, 1]

    in_maps = []
    for c in range(N_CORES):
        z_loc = z[c * B_LOC:(c + 1) * B_LOC]          # (B_LOC, 32)
        strip = np.empty((32, B_LOC), dtype=np.float32)
        strip[:31] = z_loc.T[:31]
        strip[31] = 1.0
        zt4 = np.tile(strip, (4, 1)).astype(NP_BF16)  # (128, B_LOC)
        in_maps.append({
            "zt4": zt4,
            "w1e": w1e,
            "w2": w2sb,
            "w3": w3sb,
            "b2": b2sb,
            "b3": b3sb,
        })
    return in_maps


def unmarshal_outputs(results):
    """results: per-core dicts with 'out' of shape (N_WALL, 128, B_LOC)."""
    mus = np.empty((B, L), dtype=np.float32)
    lvs = np.empty((B, L), dtype=np.float32)
    rows = np.array([32 * p + c4 for p in range(4) for c4 in range(4)])
    for c, res in enumerate(results):
        o = np.asarray(res["out"])[:, rows, :]        # (N_WALL, 16, B_LOC)
        o = o.reshape(N_WALL, 4, 2, 2, B_LOC)
        # [w, p, wl, m, b] ; l = 8w + 2p + wl
        o = np.transpose(o, (4, 0, 1, 2, 3)).reshape(B_LOC, L, 2)
        mus[c * B_LOC:(c + 1) * B_LOC] = o[:, :, 0]
        lvs[c * B_LOC:(c + 1) * B_LOC] = o[:, :, 1]
    return mus, lvs


_PROGRAM = None


def _get_program():
    global _PROGRAM
    if _PROGRAM is None:
        _PROGRAM = build_program()
    return _PROGRAM


def run(inputs, trace=False):
    nc = _get_program()
    in_maps = marshal_inputs(**inputs)
    res = run_bass_kernel_spmd(
        nc, in_maps, core_ids=list(range(N_CORES)), trace=trace)
    return unmarshal_outputs(res.results), res.exec_time_ns


def kernel(**inputs):
    out, _ = run(inputs, trace=False)
    return out


# revision 22
# speedup vs baseline: 1.0141x; 1.0141x over previous
"""nn_ARPrior kernel for 8 TRN2 NeuronCores (data-parallel over batch).

Reference computation (per batch row b, latent index l):
    inp[b,l] = 0 if l==0 else mean(z[b,:l])
    h1 = relu(inp * W1[l,0,:] + b1[l])          # (128,)
    h2 = relu(h1 @ W2[l] + b2[l])               # (64,)
    out = h2 @ W3[l] + b3[l]                    # (2,) -> (mu, logvar)
Returns (mus, lvs), each (B, L) float32.

Mapping (per core, B_LOC = 4096 batch rows):
  - inp is linear in z: inp = z @ M with M[i,l] = 1/l for i < l else 0.
    Layer 1 for latent l is a single K=32 matmul with
    lhsT[k,h] = M[k,l]*W1[l,0,h] (k<=30) and lhsT[31,h] = b1[l,h], against a
    shared moving operand [z^T rows 0..30 ; ones] (z row 31 is never needed,
    M is strictly upper triangular).
  - The moving operand is stacked 4x across SBUF partitions so 4 latents run
    concurrently on the PE array via row-tiling (tile_position=(32s,0)).
  - Layer 2: per-latent [128,64] matmul; two latents run concurrently via
    column-tiling (tile_position=(0,0)/(0,64)) into one PSUM bank.
  - Layer 3: latent pairs are packed block-diagonally into [128,4] weights;
    4 pairs run concurrently via column-tiling (tile_position=(0,32p)).
  - All matmul inputs are bf16 (fp32 accumulate in PSUM). All PE matmuls are
    chained with ordering-only deps so rotation groups stay adjacent in the
    PE stream (adjacency is what makes tile_position concurrency engage).
  - The kernel is lane-bound: ScalarE+VectorE evacuating PSUM (h1/h2/out) at
    1 elem/lane/cycle is the floor; relu/bias stages are split across the
    two engines by a cost-balancing scheduler.
  - Outputs accumulate into four persistent [128, B_LOC] wall tiles whose
    column chunks DMA out as soon as each chunk's layer-3 completes.
"""

import numpy as np
import ml_dtypes

import concourse.bass as bass
import concourse.tile as tile
from concourse import bacc, mybir
from concourse.bass_utils import run_bass_kernel_spmd

B = 32768
L = 32
H1 = 128
H2 = 64
N_CORES = 8
B_LOC = B // N_CORES          # 4096 batch rows per core
NT = 512                      # columns per matmul (fp32 PSUM bank)
N_BT = B_LOC // NT            # 8 batch tiles
N_QUAD = L // 4               # 8 quads of 4 latents
N_PAIR = L // 2               # 16 latent pairs
N_WALL = N_PAIR // 4          # 4 walls of 4 pairs

BF16 = mybir.dt.bfloat16
F32 = mybir.dt.float32
NP_BF16 = ml_dtypes.bfloat16


def build_program():
    """Build the per-core bass program (identical on all 8 cores)."""
    nc = bacc.Bacc("TRN2", target_bir_lowering=False, debug=False,
                   num_devices=N_CORES)

    d_zt4 = nc.dram_tensor("zt4", [128, B_LOC], BF16, kind="ExternalInput")
    d_w1e = nc.dram_tensor("w1e", [128, N_QUAD * H1], BF16, kind="ExternalInput")
    d_w2 = nc.dram_tensor("w2", [128, L * H2], BF16, kind="ExternalInput")
    d_w3 = nc.dram_tensor("w3", [128, N_PAIR * 4], BF16, kind="ExternalInput")
    d_b2 = nc.dram_tensor("b2", [128, N_PAIR], F32, kind="ExternalInput")
    d_b3 = nc.dram_tensor("b3", [128, N_WALL], F32, kind="ExternalInput")
    d_out = nc.dram_tensor("out", [N_WALL, 128, B_LOC], F32,
                           kind="ExternalOutput")

    # Lane-engine load balancer: assign each relu/bias op to the engine with
    # less accumulated estimated time.  Costs in ns per op (HW-measured).
    lane_time = {"dve": 0.0, "act": 0.0}

    def pick_engine(dve_cost, act_cost):
        if lane_time["dve"] + dve_cost <= lane_time["act"] + act_cost:
            lane_time["dve"] += dve_cost
            return "dve"
        lane_time["act"] += act_cost
        return "act"

    with tile.TileContext(nc) as tc:
        with (
            tc.tile_pool(name="consts", bufs=1) as consts,
            tc.tile_pool(name="h1p", bufs=10) as h1p,
            tc.tile_pool(name="h2p", bufs=16) as h2p,
            tc.tile_pool(name="outp", bufs=1) as outp,
            tc.tile_pool(name="p1", bufs=2, space="PSUM") as p1,
            tc.tile_pool(name="p2", bufs=4, space="PSUM") as p2,
        ):
            zt4 = consts.tile([128, B_LOC], BF16)
            w1e = consts.tile([128, N_QUAD * H1], BF16)
            w2 = consts.tile([128, L * H2], BF16)
            w3 = consts.tile([128, N_PAIR * 4], BF16)
            b2 = consts.tile([128, N_PAIR], F32)
            b3 = consts.tile([128, N_WALL], F32)
            nc.gpsimd.dma_start(out=w1e[:], in_=d_w1e[:])
            for _c in range(8):
                _sl = slice(_c * (B_LOC // 8), (_c + 1) * (B_LOC // 8))
                nc.gpsimd.dma_start(out=zt4[:, _sl], in_=d_zt4[:, _sl])
            nc.gpsimd.dma_start(out=w2[:], in_=d_w2[:])
            nc.gpsimd.dma_start(out=w3[:], in_=d_w3[:])
            nc.gpsimd.dma_start(out=b2[:], in_=d_b2[:])
            nc.gpsimd.dma_start(out=b3[:], in_=d_b3[:])

            # Pre-warm the ACT relu table set so its ~2.7us load overlaps
            # the input DMAs instead of delaying the first real relu.
            warm = consts.tile([1, 8], F32)
            nc.vector.memset(warm[:], 0.0)
            nc.scalar.activation(out=warm[:], in_=warm[:],
                                 func=mybir.ActivationFunctionType.Relu)

            def relu_from_psum(dst, src, force=None):
                # plain relu, PSUM(f32) -> SBUF(bf16), FD = 1024
                if force is None:
                    eng = pick_engine(dve_cost=1180.0, act_cost=1000.0)
                else:
                    eng = force
                    lane_time[eng] += 1180.0 if eng == "dve" else 1000.0
                if eng == "dve":
                    nc.vector.tensor_scalar(
                        out=dst, in0=src, scalar1=0.0, scalar2=None,
                        op0=mybir.AluOpType.max)
                else:
                    nc.scalar.activation(
                        out=dst, in_=src,
                        func=mybir.ActivationFunctionType.Relu)
                return eng

            def relu_bias_from_psum(dst, src, bias_ap):
                # relu(x + bias), PSUM(f32) -> SBUF(bf16), FD = 512
                eng = pick_engine(dve_cost=658.0, act_cost=570.0)
                if eng == "dve":
                    nc.vector.tensor_scalar(
                        out=dst, in0=src, scalar1=bias_ap, scalar2=0.0,
                        op0=mybir.AluOpType.add, op1=mybir.AluOpType.max)
                else:
                    nc.scalar.activation(
                        out=dst, in_=src,
                        func=mybir.ActivationFunctionType.Relu,
                        bias=bias_ap, scale=1.0)

            def bias_from_psum(dst, src, bias_ap):
                # x + bias, PSUM(f32) -> SBUF(f32), FD = 512
                # ACT's biased-Identity measured erratic; keep on DVE.
                eng = pick_engine(dve_cost=671.0, act_cost=100000.0)
                if eng == "dve":
                    nc.vector.tensor_scalar(
                        out=dst, in0=src, scalar1=bias_ap, scalar2=None,
                        op0=mybir.AluOpType.add)
                else:
                    nc.scalar.activation(
                        out=dst, in_=src,
                        func=mybir.ActivationFunctionType.Identity,
                        bias=bias_ap, scale=1.0)

            # Chain all PE matmuls with ordering-only deps so the scheduler
            # keeps rotation groups adjacent in the PE stream.
            pe_state = {"last": None}

            def mm(out, lhsT, rhs, tp):
                inst = nc.tensor.matmul(
                    out=out, lhsT=lhsT, rhs=rhs, start=True, stop=True,
                    tile_position=tp)
                if pe_state["last"] is not None:
                    bass._add_dep_helper(
                        inst.ins, pe_state["last"].ins, sync=False,
                        reason="pe-order")
                pe_state["last"] = inst

            def emit_l1(q, t, h1_tiles):
                col = slice(t * NT, (t + 1) * NT)
                ps_a = p1.tile([128, 2 * NT], F32, tag="p1", name=f"p1a_{q}_{t}")
                ps_b = p1.tile([128, 2 * NT], F32, tag="p1", name=f"p1b_{q}_{t}")
                for s in range(4):
                    ps = ps_a if s < 2 else ps_b
                    half = slice((s % 2) * NT, (s % 2) * NT + NT)
                    mm(ps[:, half],
                       w1e[32 * s:32 * s + 32, q * H1:(q + 1) * H1],
                       zt4[32 * s:32 * s + 32, col],
                       (32 * s, 0))
                h1_a = h1p.tile([128, 2 * NT], BF16, tag="h1", name=f"h1a_{q}_{t}")
                h1_b = h1p.tile([128, 2 * NT], BF16, tag="h1", name=f"h1b_{q}_{t}")
                relu_from_psum(h1_a[:], ps_a[:])
                relu_from_psum(h1_b[:], ps_b[:])
                h1_tiles[(q, t)] = (h1_a, h1_b)

            def emit_l2(q, t, h1_tiles, h2_tiles):
                h1_a, h1_b = h1_tiles.pop((q, t))
                for jj in range(2):
                    j = 2 * q + jj            # global pair index
                    h1t = h1_a if jj == 0 else h1_b
                    ps2 = p2.tile([128, NT], F32, tag="p2", name=f"p2_{j}_{t}")
                    for u in range(2):        # latent l = 2j + u
                        lat = 2 * j + u
                        mm(ps2[64 * u:64 * u + 64, :],
                           w2[:, H2 * lat:H2 * (lat + 1)],
                           h1t[:, u * NT:(u + 1) * NT],
                           (0, 64 * u))
                    h2t = h2p.tile([128, NT], BF16, tag="h2", name=f"h2_{j}_{t}")
                    relu_bias_from_psum(h2t[:], ps2[:], b2[:, j:j + 1])
                    h2_tiles[(j, t)] = h2t

            def emit_l3(w, t, h2_tiles, wall_tiles):
                col = slice(t * NT, (t + 1) * NT)
                ps3 = p2.tile([128, NT], F32, tag="p2", name=f"p3_{w}_{t}")
                for p in range(4):
                    j = 4 * w + p
                    mm(ps3[32 * p:32 * p + 4, :],
                       w3[:, 4 * j:4 * j + 4],
                       h2_tiles.pop((j, t))[:],
                       (0, 32 * p))
                bias_from_psum(wall_tiles[w][:, col], ps3[:], b3[:, w:w + 1])
                nc.gpsimd.dma_start(out=d_out[w, :, col],
                                    in_=wall_tiles[w][:, col])

            # Software-pipelined emission: L2 lags L1 by one step, L3 lags L2.
            wall_tiles = [
                outp.tile([128, B_LOC], F32, name=f"wall{w}")
                for w in range(N_WALL)
            ]
            steps = [(q, t) for t in range(N_BT) for q in range(N_QUAD)]
            h1_tiles, h2_tiles = {}, {}
            prev = None          # (q, t) whose L2 is pending
            prev_l3 = None       # (w, t) whose L3 is pending
            for (q, t) in steps:
                emit_l1(q, t, h1_tiles)
                if prev is not None:
                    pq, pt = prev
                    emit_l2(pq, pt, h1_tiles, h2_tiles)
                    if prev_l3 is not None:
                        emit_l3(*prev_l3, h2_tiles, wall_tiles)
                        prev_l3 = None
                    if pq % 2 == 1:
                        prev_l3 = (pq // 2, pt)
                prev = (q, t)
            pq, pt = prev
            emit_l2(pq, pt, h1_tiles, h2_tiles)
            if prev_l3 is not None:
                emit_l3(*prev_l3, h2_tiles, wall_tiles)
            emit_l3(pq // 2, pt, h2_tiles, wall_tiles)

    nc.compile()
    return nc


def marshal_inputs(z, W1, b1, W2, b2, W3, b3):
    """Build the 8 per-core input maps from full fp32 inputs."""
    z = np.asarray(z, dtype=np.float32)
    W1 = np.asarray(W1, dtype=np.float64)
    b1 = np.asarray(b1, dtype=np.float64)
    W2 = np.asarray(W2, dtype=np.float32)
    b2 = np.asarray(b2, dtype=np.float32)
    W3 = np.asarray(W3, dtype=np.float32)
    b3 = np.asarray(b3, dtype=np.float32)

    # Cumsum/mean fold: M[i,l] = 1/l for i < l else 0 (col 0 = zeros).
    M = np.zeros((L, L), dtype=np.float64)
    for l in range(1, L):
        M[:l, l] = 1.0 / l

    # W1eff[l, k, h]: k<=30 -> M[k,l] * W1[l,0,h]; k==31 -> b1[l,h].
    w1eff = np.einsum("kl,lh->lkh", M, W1[:, 0, :])   # (L, 32, 128)
    w1eff[:, 31, :] = b1                              # row 31 of M is all zero
    # pack: w1e[32s+k, 128q+h] = w1eff[4q+s, k, h]
    w1e = np.zeros((128, N_QUAD * H1), dtype=np.float64)
    for q in range(N_QUAD):
        for s in range(4):
            w1e[32 * s:32 * s + 32, q * H1:(q + 1) * H1] = w1eff[4 * q + s]
    w1e = w1e.astype(NP_BF16)

    # w2[h, 64l+o] = W2[l, h, o]
    w2sb = np.transpose(W2, (1, 0, 2)).reshape(H1, L * H2).astype(NP_BF16)

    # w3 block-diag pairs: [128, 4 per pair]
    w3sb = np.zeros((128, N_PAIR * 4), dtype=np.float32)
    for j in range(N_PAIR):
        w3sb[0:64, 4 * j + 0] = W3[2 * j, :, 0]
        w3sb[0:64, 4 * j + 1] = W3[2 * j, :, 1]
        w3sb[64:128, 4 * j + 2] = W3[2 * j + 1, :, 0]
        w3sb[64:128, 4 * j + 3] = W3[2 * j + 1, :, 1]
    w3sb = w3sb.astype(NP_BF16)

    # b2sb[o, j] = b2[2j, o]; b2sb[64+o, j] = b2[2j+1, o]
    b2sb = np.zeros((128, N_PAIR), dtype=np.float32)
    for j in range(N_PAIR):
        b2sb[0:64, j] = b2[2 * j]
        b2sb[64:128, j] = b2[2 * j + 1]

    # b3sb[32p + (2*wl + m), w] = b3[8w + 2p + wl, m]
    b3sb = np.zeros((128, N_WALL), dtype=np.float32)
    for w in range(N_WALL):
        for p in range(4):
            for wl in range(2):
                lat = 8 * w + 2 * p + wl
                b3sb[32 * p + 2 * wl + 0, w] = b3[lat, 0]
                b3sb[32 * p + 2 * wl + 1, w] = b3[lat, 1]

    in_maps = []
    for c in range(N_CORES):
        z_loc = z[c * B_LOC:(c + 1) * B_LOC]          # (B_LOC, 32)
        strip = np.empty((32, B_LOC), dtype=np.float32)
        strip[:31] = z_loc.T[:31]
        strip[31] = 1.0
        zt4 = np.tile(strip, (4, 1)).astype(NP_BF16)  # (128, B_LOC)
        in_maps.append({
            "zt4": zt4,
            "w1e": w1e,
            "w2": w2sb,
            "w3": w3sb,
            "b2": b2sb,
            "b3": b3sb,
        })
    return in_maps


def unmarshal_outputs(results):
    """results: per-core dicts with 'out' of shape (N_WALL, 128, B_LOC)."""
    mus = np.empty((B, L), dtype=np.float32)
    lvs = np.empty((B, L), dtype=np.float32)
    rows = np.array([32 * p + c4 for p in range(4) for c4 in range(4)])
    for c, res in enumerate(results):
        o = np.asarray(res["out"])[:, rows, :]        # (N_WALL, 16, B_LOC)
        o = o.reshape(N_WALL, 4, 2, 2, B_LOC)
        # [w, p, wl, m, b] ; l = 8w + 2p + wl
        o = np.transpose(o, (4, 0, 1, 2, 3)).reshape(B_LOC, L, 2)
        mus[c * B_LOC:(c + 1) * B_LOC] = o[:, :, 0]
        lvs[c * B_LOC:(c + 1) * B_LOC] = o[:, :, 1]
    return mus, lvs


_PROGRAM = None


def _get_program():
    global _PROGRAM
    if _PROGRAM is None:
        _PROGRAM = build_program()
    return _PROGRAM


def run(inputs, trace=False):
    nc = _get_program()
    in_maps = marshal_inputs(**inputs)
    res = run_bass_kernel_spmd(
        nc, in_maps, core_ids=list(range(N_CORES)), trace=trace)
    return unmarshal_outputs(res.results), res.exec_time_ns


def kernel(**inputs):
    out, _ = run(inputs, trace=False)
    return out
